# revision 16
# baseline (speedup 1.0000x reference)
"""DVH global loss (histogram binning) Trainium2 kernel, v2.

Strategy: 8 cores, data-parallel over (batch, voxel-half): core = 2*b + h.
Each core bins 2x 1M voxels (pred, gt) into a 16x32 (q, r) joint histogram,
q = j>>5, r = j&31, j = searchsorted(linspace(0,75,500), d*m, 'right') - 1
computed with exact fp32 magic-number rounding chains.

Engine split (vs v1 which was DVE-bound at 1x broadcast tensor_tensor):
  - index chain: dual-scalar tensor_scalar ops on DVE (fp32 2x_2P mode) +
    one ACT op (v = 32*qm + bias, fma-exact).
  - one-hots: per-row tensor_scalar is_equal (bf16, step-1, even dim ->
    4x_2P mode, 4 elem/cycle/lane), q rows 0..15 and r rows 0..(31-K_TAU)
    on DVE; the last K_TAU r-rows are +-1 thermometer rows built on the
    otherwise-idle ACT engine via Sign(r - (b-0.5)) (decoded on host by
    tail-differencing; exact integer algebra).
  - PE: 8 voxel-columns per matmul via block-strided APs: stationary =
    ah[:, 16a, 8v] (128 cols), moving = bh[:, 32b, 8v] (256 cols), all
    matmuls of one tensor accumulate into a single persistent PSUM
    region [128, 256]; host extracts the 8 diagonal (v == v') blocks.
    2048 LDWEIGHTS+MATMUL pairs total vs 32768 in v1.

Masked-out voxels are shifted past bin 4000 so their q misses the 16-wide
one-hot (their r rows are nonzero but always multiply ah == 0).
mask-sum is recovered on host as the total count in the pred histogram.

A post-Tile pass legalizes semaphore waits (trn2 wait-slot limits).
"""

import sys
from contextlib import ExitStack

if "/opt/trn_rl_repo" not in sys.path:
    sys.path.insert(0, "/opt/trn_rl_repo")

import numpy as np

import concourse.bass as bass
import concourse.tile as tile
from concourse import mybir
from concourse.bass_utils import run_bass_kernel_spmd

F32 = mybir.dt.float32
BF16 = mybir.dt.bfloat16

C1 = 499.0 / 75.0
GUARD = 0.4998
U2_S1 = -4000.0 / C1  # * m
U2_S2 = (4000.0 - GUARD) / C1  # + const
M1 = 12582912.0  # 1.5 * 2^23 fp32 round-to-int magic
M2 = 12582912.0


# trn2 engine instructions have very few sync-wait slots (TT has one). Tile
# emits redundant same-engine waits and multi-waits that walrus rejects.
# Legalize: drop own-engine-sem waits on in-order compute engines, then move
# excess waits onto earlier same-engine instructions with free slots.
_ENGINE_SEM_PREFIX = {
    mybir.EngineType.DVE: "DVE_",
    mybir.EngineType.Activation: "Activation_",
    mybir.EngineType.Pool: "Pool_",
}

_EXEMPT_TYPES = (
    "InstCall",
    "InstUnconditionalBranch",
    "InstRegisterMove",
    "InstISA",
    "InstNoOp",
)

_SELF_DROP_TYPES = (
    "InstTensorTensor",
    "InstTensorScalarPtr",
    "InstTensorScalar",
    "InstTensorReduce",
    "InstActivation",
    "InstMemset",
    "InstTensorCopy",
)


def legalize_sync_waits(nc, max_waits=1):
    """trn2 engine instructions have very few sync-wait slots (TT and DMA
    structs have one). Drop redundant same-engine waits on in-order compute
    engines, then split remaining excess waits onto same-engine NOPs
    inserted immediately before the instruction."""
    eng_map = {
        mybir.EngineType.DVE: nc.vector,
        mybir.EngineType.Activation: nc.scalar,
        mybir.EngineType.Pool: nc.gpsimd,
        mybir.EngineType.PE: nc.tensor,
        mybir.EngineType.SP: nc.sync,
    }
    for fn in nc.m.functions:
        blocks = list(fn.blocks)
        for blk in blocks:
            insts = blk.instructions
            work = []
            for i, ins in enumerate(insts):
                tname = type(ins).__name__
                if tname in _EXEMPT_TYPES:
                    continue
                si = ins.sync_info
                if si is None:
                    continue
                waits = list(si.on_wait)
                eng = ins.engine
                pref = _ENGINE_SEM_PREFIX.get(eng)
                if pref is not None and tname in _SELF_DROP_TYPES:
                    waits = [
                        w for w in waits
                        if not (w.ant_name or "").startswith(pref)
                    ]
                if len(waits) == len(si.on_wait) and len(waits) <= max_waits:
                    continue
                work.append((i, ins, waits))
            for i, ins, waits in reversed(work):
                si = ins.sync_info
                keep, excess = waits[:max_waits], waits[max_waits:]
                ins.sync_info = mybir.SyncInfo(
                    on_wait=keep, on_update=si.on_update
                )
                eng_iface = eng_map[ins.engine]
                for w in reversed(excess):
                    bi = eng_iface.nop(nofuse=True)
                    mi = bi.ins
                    for b2 in fn.blocks:
                        L = b2.instructions
                        for k in range(len(L) - 1, -1, -1):
                            if L[k] is mi or L[k].name == mi.name:
                                del L[k]
                                break
                        else:
                            continue
                        break
                    mi.sync_info = mybir.SyncInfo(on_wait=[w], on_update=[])
                    blk.instructions.insert(i, mi)


def _chunk_list(FPP, V):
    """Split [0, FPP) into chunks of V (last chunk may be smaller,
    multiple of 8)."""
    out = []
    off = 0
    while off < FPP:
        v = min(V, FPP - off)
        assert v % 8 == 0
        out.append((off, v))
        off += v
    return out


def build_kernel(P=128, FPP=8192, V=768, QW=16, RW=32, K_TAU=14, B=8):
    AluOp = mybir.AluOpType
    Act = mybir.ActivationFunctionType
    chunks = _chunk_list(FPP, V)
    n_mm = sum(v // B for _, v in chunks)  # matmuls per tensor

    nc = bass.Bass()
    d_p_ext = nc.declare_dram_parameter("d_pred", [P, FPP], F32, isOutput=False)
    d_g_ext = nc.declare_dram_parameter("d_gt", [P, FPP], F32, isOutput=False)
    m_ext = nc.declare_dram_parameter("mask", [P, FPP], F32, isOutput=False)
    hp_ext = nc.declare_dram_parameter("hist_p", [P, QW * B * 2], F32, isOutput=True)
    hg_ext = nc.declare_dram_parameter("hist_g", [P, QW * B * 2], F32, isOutput=True)

    NOUT = RW * B  # 256 psum columns

    with tile.TileContext(nc) as tc, ExitStack() as ctx:
        singles = ctx.enter_context(tc.tile_pool(name="singles", bufs=1))
        ins = ctx.enter_context(tc.tile_pool(name="ins", bufs=2))
        # chain tiles are produced+consumed in program order on DVE/ACT --
        # single-buffered; q_bf/r_bf feed the (long) one-hot stage and ACT
        # Sign rows, so they get 2 buffers for cross-pass overlap.
        mids = ctx.enter_context(tc.tile_pool(name="mids", bufs=1))
        qrs = ctx.enter_context(tc.tile_pool(name="qrs", bufs=2))
        hots = ctx.enter_context(tc.tile_pool(name="hots", bufs=2))
        psums = ctx.enter_context(
            tc.tile_pool(name="psums", bufs=1, space=bass.MemorySpace.PSUM)
        )

        psum_acc = {
            "p": psums.tile([P, NOUT], F32, name="psum_p", tag="psum_p"),
            "g": psums.tile([P, NOUT], F32, name="psum_g", tag="psum_g"),
        }
        flush_sb = {
            "p": singles.tile([P, NOUT], F32, name="flush_p", tag="flush_p"),
            "g": singles.tile([P, NOUT], F32, name="flush_g", tag="flush_g"),
        }
        mm_count = {"p": 0, "g": 0}

        # per-partition bias columns for the ACT Sign thermometer rows
        if K_TAU > 0:
            tau_bias = singles.tile([P, K_TAU], F32, name="tau_bias", tag="tau_bias")
            for j, b in enumerate(range(RW - K_TAU, RW)):
                nc.vector.memset(tau_bias[:, j : j + 1], 0.5 - float(b))

        for ci, (off, v) in enumerate(chunks):
            sl = slice(off, off + v)
            d_p = ins.tile([P, V], F32, tag="d_p")
            d_g = ins.tile([P, V], F32, tag="d_g")
            m = ins.tile([P, V], F32, tag="m")
            nc.sync.dma_start(out=d_p[:, :v], in_=d_p_ext[:, sl])
            nc.sync.dma_start(out=d_g[:, :v], in_=d_g_ext[:, sl])
            nc.sync.dma_start(out=m[:, :v], in_=m_ext[:, sl])

            # u = m*U2_S1 + U2_S2, in-place over the mask tile
            # (masked-out voxels get a +4000-bin dose shift)
            u = m
            nc.vector.tensor_scalar(
                out=u[:, :v], in0=m[:, :v],
                scalar1=U2_S1, scalar2=U2_S2,
                op0=AluOp.mult, op1=AluOp.add,
            )

            for which, d_t in (("p", d_p), ("g", d_g)):
                acc = psum_acc[which]
                # in-place chain in the d tile:
                # x2 = d + u  (fp32 TT, 1x)
                nc.vector.tensor_tensor(
                    out=d_t[:, :v], in0=d_t[:, :v], in1=u[:, :v], op=AluOp.add
                )
                # t = RN(RN(x2*C1) + M1) = j + M1  (magic round; 2x_2P)
                t = mids.tile([P, V], F32, tag="t")
                nc.vector.tensor_scalar(
                    out=t[:, :v], in0=d_t[:, :v],
                    scalar1=C1, scalar2=M1,
                    op0=AluOp.mult, op1=AluOp.add,
                )
                # f1 = t/32 - 393216 = j/32 exactly (in-place over d tile)
                f1 = d_t
                nc.vector.tensor_scalar(
                    out=f1[:, :v], in0=t[:, :v],
                    scalar1=0.03125, scalar2=-393216.0,
                    op0=AluOp.mult, op1=AluOp.add,
                )
                # q_bf2 = bf16(RN(j/32 + 191.515625)) = q + 192 via the bf16
                # cast itself (ulp 1 on [192,256); offset -0.484375 keeps all
                # residues strictly inside the round-to-q window).
                q_bf = qrs.tile([P, V], BF16, tag="q_bf")
                nc.vector.tensor_scalar(
                    out=q_bf[:, :v], in0=f1[:, :v],
                    scalar1=191.515625, scalar2=None, op0=AluOp.add,
                )
                # v32 = 32*(q+192) + (M1 - 6144) = 32q + M1 (ACT fma, exact)
                v32 = mids.tile([P, V], F32, tag="v32")
                nc.scalar.activation(
                    out=v32[:, :v], in_=q_bf[:, :v], func=Act.Copy,
                    bias=M1 - 6144.0, scale=32.0,
                )
                # r_bf = t - v32 = j - 32q (bf16)
                r_bf = qrs.tile([P, V], BF16, tag="r_bf")
                nc.vector.tensor_tensor(
                    out=r_bf[:, :v], in0=t[:, :v], in1=v32[:, :v],
                    op=AluOp.subtract,
                )

                # one-hots (DVE tensor_scalar is_equal, bf16 4x mode),
                # written in PE-ready transposed layout: col = a*B + v so
                # each matmul operand is one contiguous free dim.
                vg = v // B
                q_g = q_bf[:, :v].rearrange("p (g w) -> p g w", w=B)
                r_g = r_bf[:, :v].rearrange("p (g w) -> p g w", w=B)
                ah = hots.tile([P, V // B, QW * B], BF16, tag="ah")
                for a in range(QW):
                    nc.vector.tensor_scalar(
                        out=ah[:, :vg, a * B : (a + 1) * B], in0=q_g,
                        scalar1=float(192 + a), scalar2=None,
                        op0=AluOp.is_equal,
                    )
                bh = hots.tile([P, V // B, RW * B], BF16, tag="bh")
                for b in range(RW - K_TAU):
                    nc.vector.tensor_scalar(
                        out=bh[:, :vg, b * B : (b + 1) * B], in0=r_g,
                        scalar1=float(b), scalar2=None, op0=AluOp.is_equal,
                    )
                # last K_TAU rows: +-1 thermometer via ACT Sign(r - (b-0.5))
                for b in range(RW - K_TAU, RW):
                    j = b - (RW - K_TAU)
                    nc.scalar.activation(
                        out=bh[:, :vg, b * B : (b + 1) * B], in_=r_g,
                        func=Act.Sign,
                        bias=tau_bias[:, j : j + 1], scale=1.0,
                    )

                # PE: 8 voxel-columns per matmul into persistent PSUM
                for g0 in range(vg):
                    i = mm_count[which]
                    nc.tensor.matmul(
                        acc[:, :],
                        ah[:, g0, :],  # [P, 128] stationary, contiguous
                        bh[:, g0, :],  # [P, 256] moving, contiguous
                        start=(i == 0), stop=(i == n_mm - 1),
                    )
                    mm_count[which] += 1

        for which in ("p", "g"):
            nc.vector.tensor_scalar(
                out=flush_sb[which], in0=psum_acc[which],
                scalar1=0.0, scalar2=None, op0=AluOp.add,
            )
        nc.sync.dma_start(out=hp_ext[:], in_=flush_sb["p"])
        nc.sync.dma_start(out=hg_ext[:], in_=flush_sb["g"])

    legalize_sync_waits(nc)
    return nc


NCORES = 8
P = 128
FPP = 8192  # voxels per partition per core (half a 128^3 volume / 128)
QW, RW = 16, 32
K_TAU = 15
V = 896
B = 8

_CACHE = {}


def _get_nc():
    if "nc" not in _CACHE:
        _CACHE["nc"] = build_kernel(
            P=P, FPP=FPP, V=V, QW=QW, RW=RW, K_TAU=K_TAU, B=B
        )
    return _CACHE["nc"]


def decode_hist(M, qw=QW, rw=RW, b=B, k_tau=K_TAU):
    """[128, 256] psum -> [16, 32] integer histogram (float64).

    M[a*8+v, bb*8+v'] : diagonal v==v' blocks hold sum_n ah[a]*row_b[n].
    Rows < rw-k_tau are one-hots; rows >= are +-1 thermometers
    (2*Tail - Cnt)."""
    M4 = M.astype(np.float64).reshape(qw, b, rw, b)
    Hj = np.einsum("avbv->ab", M4)  # [16, 32]
    n_oh = rw - k_tau
    H = np.zeros((qw, rw), np.float64)
    H[:, :n_oh] = Hj[:, :n_oh]
    if k_tau > 0:
        val = Hj[:, n_oh:]  # [16, k_tau]
        cnt = 2.0 * H[:, :n_oh].sum(axis=1) + val[:, 0]  # [16]
        tail = 0.5 * (val + cnt[:, None])  # [16, k_tau]
        H[:, n_oh:-1] = tail[:, :-1] - tail[:, 1:]
        H[:, -1] = tail[:, -1]
    return H


def run_device(d_pred, d_gt, mask, trace=False, tmpdir=None):
    """Run the SPMD kernel; returns (results_list, exec_time_ns)."""
    Bt = d_pred.shape[0]
    Vx = int(np.prod(d_pred.shape[1:]))
    dp = np.ascontiguousarray(d_pred, dtype=np.float32).reshape(Bt, Vx)
    dg = np.ascontiguousarray(d_gt, dtype=np.float32).reshape(Bt, Vx)
    mm = np.ascontiguousarray(mask, dtype=np.float32).reshape(Bt, Vx)
    half = Vx // 2
    in_maps = []
    for core in range(NCORES):
        bb, h = divmod(core, 2)
        sl = slice(h * half, (h + 1) * half)
        in_maps.append(
            {
                "d_pred": dp[bb, sl].reshape(P, FPP),
                "d_gt": dg[bb, sl].reshape(P, FPP),
                "mask": mm[bb, sl].reshape(P, FPP),
            }
        )
    res = run_bass_kernel_spmd(
        _get_nc(), in_maps, list(range(NCORES)), trace=trace, tmpdir=tmpdir
    )
    return res.results, res.exec_time_ns


def kernel(d_pred, d_gt, mask):
    results, _ = run_device(d_pred, d_gt, mask)
    Bt = d_pred.shape[0]
    loss = 0.0
    for bb in range(Bt):
        e = np.zeros((QW, RW), np.float64)
        msum = 0.0
        for h in range(2):
            r = results[2 * bb + h]
            hp = decode_hist(r["hist_p"])
            hg = decode_hist(r["hist_g"])
            e += hp - hg
            msum += float(hp.sum())
        ed = e.reshape(QW * RW)[:500]
        T = np.cumsum(ed[::-1])[::-1]
        denom = msum + 1e-6
        loss += float(np.sum((T / denom) ** 2))
    loss /= Bt * 500
    return np.float32(loss)


# revision 20
# speedup vs baseline: 1.1110x; 1.1110x over previous
"""DVH global loss (histogram binning) Trainium2 kernel, v2.

Strategy: 8 cores, data-parallel over (batch, voxel-half): core = 2*b + h.
Each core bins 2x 1M voxels (pred, gt) into a 16x32 (q, r) joint histogram,
q = j>>5, r = j&31, j = searchsorted(linspace(0,75,500), d*m, 'right') - 1
computed with exact fp32 magic-number rounding chains.

Engine split (vs v1 which was DVE-bound at 1x broadcast tensor_tensor):
  - index chain: dual-scalar tensor_scalar ops on DVE (fp32 2x_2P mode) +
    one ACT op (v = 32*qm + bias, fma-exact).
  - one-hots: per-row tensor_scalar is_equal (bf16, step-1, even dim ->
    4x_2P mode, 4 elem/cycle/lane), q rows 0..15 and r rows 0..(31-K_TAU)
    on DVE; the last K_TAU r-rows are +-1 thermometer rows built on the
    otherwise-idle ACT engine via Sign(r - (b-0.5)) (decoded on host by
    tail-differencing; exact integer algebra).
  - PE: 8 voxel-columns per matmul via block-strided APs: stationary =
    ah[:, 16a, 8v] (128 cols), moving = bh[:, 32b, 8v] (256 cols), all
    matmuls of one tensor accumulate into a single persistent PSUM
    region [128, 256]; host extracts the 8 diagonal (v == v') blocks.
    2048 LDWEIGHTS+MATMUL pairs total vs 32768 in v1.

Masked-out voxels are shifted past bin 4000 so their q misses the 16-wide
one-hot (their r rows are nonzero but always multiply ah == 0).
mask-sum is recovered on host as the total count in the pred histogram.

A post-Tile pass legalizes semaphore waits (trn2 wait-slot limits).
"""

import sys
from contextlib import ExitStack

if "/opt/trn_rl_repo" not in sys.path:
    sys.path.insert(0, "/opt/trn_rl_repo")

import numpy as np

import concourse.bass as bass
import concourse.tile as tile
from concourse import mybir
from concourse.bass_utils import run_bass_kernel_spmd

F32 = mybir.dt.float32
BF16 = mybir.dt.bfloat16

C1 = 499.0 / 75.0
GUARD = 0.4998
U2_S1 = -4000.0 / C1  # * m
U2_S2 = (4000.0 - GUARD) / C1  # + const
M1 = 12582912.0  # 1.5 * 2^23 fp32 round-to-int magic
M2 = 12582912.0


# trn2 engine instructions have very few sync-wait slots (TT has one). Tile
# emits redundant same-engine waits and multi-waits that walrus rejects.
# Legalize: drop own-engine-sem waits on in-order compute engines, then move
# excess waits onto earlier same-engine instructions with free slots.
_ENGINE_SEM_PREFIX = {
    mybir.EngineType.DVE: "DVE_",
    mybir.EngineType.Activation: "Activation_",
    mybir.EngineType.Pool: "Pool_",
}

_EXEMPT_TYPES = (
    "InstCall",
    "InstUnconditionalBranch",
    "InstRegisterMove",
    "InstISA",
    "InstNoOp",
)

_SELF_DROP_TYPES = (
    "InstTensorTensor",
    "InstTensorScalarPtr",
    "InstTensorScalar",
    "InstTensorReduce",
    "InstActivation",
    "InstMemset",
    "InstTensorCopy",
)


def legalize_sync_waits(nc, max_waits=1):
    """trn2 engine instructions have very few sync-wait slots (TT and DMA
    structs have one). Drop redundant same-engine waits on in-order compute
    engines, then split remaining excess waits onto same-engine NOPs
    inserted immediately before the instruction."""
    eng_map = {
        mybir.EngineType.DVE: nc.vector,
        mybir.EngineType.Activation: nc.scalar,
        mybir.EngineType.Pool: nc.gpsimd,
        mybir.EngineType.PE: nc.tensor,
        mybir.EngineType.SP: nc.sync,
    }
    for fn in nc.m.functions:
        blocks = list(fn.blocks)
        for blk in blocks:
            insts = blk.instructions
            work = []
            for i, ins in enumerate(insts):
                tname = type(ins).__name__
                if tname in _EXEMPT_TYPES:
                    continue
                si = ins.sync_info
                if si is None:
                    continue
                waits = list(si.on_wait)
                eng = ins.engine
                pref = _ENGINE_SEM_PREFIX.get(eng)
                if pref is not None and tname in _SELF_DROP_TYPES:
                    waits = [
                        w for w in waits
                        if not (w.ant_name or "").startswith(pref)
                    ]
                if len(waits) == len(si.on_wait) and len(waits) <= max_waits:
                    continue
                work.append((i, ins, waits))
            for i, ins, waits in reversed(work):
                si = ins.sync_info
                keep, excess = waits[:max_waits], waits[max_waits:]
                ins.sync_info = mybir.SyncInfo(
                    on_wait=keep, on_update=si.on_update
                )
                eng_iface = eng_map[ins.engine]
                for w in reversed(excess):
                    bi = eng_iface.nop(nofuse=True)
                    mi = bi.ins
                    for b2 in fn.blocks:
                        L = b2.instructions
                        for k in range(len(L) - 1, -1, -1):
                            if L[k] is mi or L[k].name == mi.name:
                                del L[k]
                                break
                        else:
                            continue
                        break
                    mi.sync_info = mybir.SyncInfo(on_wait=[w], on_update=[])
                    blk.instructions.insert(i, mi)


def _chunk_list(FPP, V):
    """Split [0, FPP) into chunks of V (last chunk may be smaller,
    multiple of 8)."""
    out = []
    off = 0
    while off < FPP:
        v = min(V, FPP - off)
        assert v % 8 == 0
        out.append((off, v))
        off += v
    return out


def build_kernel(P=128, FPP=8192, V=768, QW=16, RW=32, K_TAU=14, B=8):
    AluOp = mybir.AluOpType
    Act = mybir.ActivationFunctionType
    chunks = _chunk_list(FPP, V)
    n_mm = sum(v // B for _, v in chunks)  # matmuls per tensor

    nc = bass.Bass()
    d_p_ext = nc.declare_dram_parameter("d_pred", [P, FPP], F32, isOutput=False)
    d_g_ext = nc.declare_dram_parameter("d_gt", [P, FPP], F32, isOutput=False)
    m_ext = nc.declare_dram_parameter("mask", [P, FPP], F32, isOutput=False)
    hp_ext = nc.declare_dram_parameter("hist_p", [P, QW * B * 2], F32, isOutput=True)
    hg_ext = nc.declare_dram_parameter("hist_g", [P, QW * B * 2], F32, isOutput=True)

    NOUT = RW * B  # 256 psum columns

    with tile.TileContext(nc) as tc, ExitStack() as ctx:
        singles = ctx.enter_context(tc.tile_pool(name="singles", bufs=1))
        ins = ctx.enter_context(tc.tile_pool(name="ins", bufs=2))
        # chain tiles are produced+consumed in program order on DVE/ACT --
        # single-buffered; q_bf/r_bf feed the (long) one-hot stage and ACT
        # Sign rows, so they get 2 buffers for cross-pass overlap.
        mids = ctx.enter_context(tc.tile_pool(name="mids", bufs=1))
        qrs = ctx.enter_context(tc.tile_pool(name="qrs", bufs=2))
        hots = ctx.enter_context(tc.tile_pool(name="hots", bufs=2))
        psums = ctx.enter_context(
            tc.tile_pool(name="psums", bufs=1, space=bass.MemorySpace.PSUM)
        )

        psum_acc = {
            "p": psums.tile([P, NOUT], F32, name="psum_p", tag="psum_p"),
            "g": psums.tile([P, NOUT], F32, name="psum_g", tag="psum_g"),
        }
        flush_sb = {
            "p": singles.tile([P, NOUT], F32, name="flush_p", tag="flush_p"),
            "g": singles.tile([P, NOUT], F32, name="flush_g", tag="flush_g"),
        }
        mm_count = {"p": 0, "g": 0}

        # per-partition bias columns for the ACT Sign thermometer rows
        if K_TAU > 0:
            tau_bias = singles.tile([P, K_TAU], F32, name="tau_bias", tag="tau_bias")
            for j, b in enumerate(range(RW - K_TAU, RW)):
                nc.vector.memset(tau_bias[:, j : j + 1], 0.5 - float(b))

        for ci, (off, v) in enumerate(chunks):
            sl = slice(off, off + v)
            d_p = ins.tile([P, V], F32, tag="d_p")
            d_g = ins.tile([P, V], F32, tag="d_g")
            m = ins.tile([P, V], F32, tag="m")
            nc.sync.dma_start(out=d_p[:, :v], in_=d_p_ext[:, sl])
            nc.sync.dma_start(out=d_g[:, :v], in_=d_g_ext[:, sl])
            nc.sync.dma_start(out=m[:, :v], in_=m_ext[:, sl])

            # u = m*U2_S1 + U2_S2, in-place over the mask tile
            # (masked-out voxels get a +4000-bin dose shift)
            u = m
            nc.vector.tensor_scalar(
                out=u[:, :v], in0=m[:, :v],
                scalar1=U2_S1, scalar2=U2_S2,
                op0=AluOp.mult, op1=AluOp.add,
            )

            for which, d_t in (("p", d_p), ("g", d_g)):
                acc = psum_acc[which]
                # in-place chain in the d tile:
                # x2 = d + u  (fp32 TT, 1x)
                nc.vector.tensor_tensor(
                    out=d_t[:, :v], in0=d_t[:, :v], in1=u[:, :v], op=AluOp.add
                )
                # t = RN(RN(x2*C1) + M1) = j + M1  (magic round; 2x_2P),
                # then f1 = t/32 - 393216 = j/32 exactly; both in-place.
                nc.vector.tensor_scalar(
                    out=d_t[:, :v], in0=d_t[:, :v],
                    scalar1=C1, scalar2=M1,
                    op0=AluOp.mult, op1=AluOp.add,
                )
                f1 = d_t
                nc.vector.tensor_scalar(
                    out=f1[:, :v], in0=d_t[:, :v],
                    scalar1=0.03125, scalar2=-393216.0,
                    op0=AluOp.mult, op1=AluOp.add,
                )
                # q_bf2 = bf16(RN(j/32 + 191.515625)) = q + 192 via the bf16
                # cast itself (ulp 1 on [192,256); offset -0.484375 keeps all
                # residues strictly inside the round-to-q window).
                q_bf = qrs.tile([P, V], BF16, tag="q_bf")
                nc.vector.tensor_scalar(
                    out=q_bf[:, :v], in0=f1[:, :v],
                    scalar1=191.515625, scalar2=None, op0=AluOp.add,
                )
                # r2 = (q_bf - 192) - f1 = -r/32 exactly (bf16); one DVE op,
                # no ACT round-trip. bh one-hot iotas are -b/32.
                r_bf = qrs.tile([P, V], BF16, tag="r_bf")
                nc.vector.scalar_tensor_tensor(
                    out=r_bf[:, :v], in0=q_bf[:, :v], scalar=192.0,
                    in1=f1[:, :v],
                    op0=AluOp.subtract, op1=AluOp.subtract,
                )

                # one-hots (DVE tensor_scalar is_equal, bf16 4x mode),
                # written in PE-ready transposed layout: col = a*B + v so
                # each matmul operand is one contiguous free dim.
                vg = v // B
                q_g = q_bf[:, :v].rearrange("p (g w) -> p g w", w=B)
                r_g = r_bf[:, :v].rearrange("p (g w) -> p g w", w=B)
                ah = hots.tile([P, V // B, QW * B], BF16, tag="ah")
                for a in range(QW):
                    nc.vector.tensor_scalar(
                        out=ah[:, :vg, a * B : (a + 1) * B], in0=q_g,
                        scalar1=float(192 + a), scalar2=None,
                        op0=AluOp.is_equal,
                    )
                bh = hots.tile([P, V // B, RW * B], BF16, tag="bh")
                for b in range(RW - K_TAU):
                    nc.vector.tensor_scalar(
                        out=bh[:, :vg, b * B : (b + 1) * B], in0=r_g,
                        scalar1=-b * 0.03125, scalar2=None, op0=AluOp.is_equal,
                    )
                # last K_TAU rows: +-1 thermometer via ACT
                # Sign(-32*r2 - (b-0.5)) = Sign(r - b + 0.5)
                for b in range(RW - K_TAU, RW):
                    j = b - (RW - K_TAU)
                    nc.scalar.activation(
                        out=bh[:, :vg, b * B : (b + 1) * B], in_=r_g,
                        func=Act.Sign,
                        bias=tau_bias[:, j : j + 1], scale=-32.0,
                    )

                # PE: 8 voxel-columns per matmul into persistent PSUM
                for g0 in range(vg):
                    i = mm_count[which]
                    nc.tensor.matmul(
                        acc[:, :],
                        ah[:, g0, :],  # [P, 128] stationary, contiguous
                        bh[:, g0, :],  # [P, 256] moving, contiguous
                        start=(i == 0), stop=(i == n_mm - 1),
                    )
                    mm_count[which] += 1

        for which in ("p", "g"):
            nc.vector.tensor_scalar(
                out=flush_sb[which], in0=psum_acc[which],
                scalar1=0.0, scalar2=None, op0=AluOp.add,
            )
        nc.sync.dma_start(out=hp_ext[:], in_=flush_sb["p"])
        nc.sync.dma_start(out=hg_ext[:], in_=flush_sb["g"])

    legalize_sync_waits(nc)
    return nc


NCORES = 8
P = 128
FPP = 8192  # voxels per partition per core (half a 128^3 volume / 128)
QW, RW = 16, 32
K_TAU = 16
V = 928
B = 8

_CACHE = {}


def _get_nc():
    if "nc" not in _CACHE:
        _CACHE["nc"] = build_kernel(
            P=P, FPP=FPP, V=V, QW=QW, RW=RW, K_TAU=K_TAU, B=B
        )
    return _CACHE["nc"]


def decode_hist(M, qw=QW, rw=RW, b=B, k_tau=K_TAU):
    """[128, 256] psum -> [16, 32] integer histogram (float64).

    M[a*8+v, bb*8+v'] : diagonal v==v' blocks hold sum_n ah[a]*row_b[n].
    Rows < rw-k_tau are one-hots; rows >= are +-1 thermometers
    (2*Tail - Cnt)."""
    M4 = M.astype(np.float64).reshape(qw, b, rw, b)
    Hj = np.einsum("avbv->ab", M4)  # [16, 32]
    n_oh = rw - k_tau
    H = np.zeros((qw, rw), np.float64)
    H[:, :n_oh] = Hj[:, :n_oh]
    if k_tau > 0:
        val = Hj[:, n_oh:]  # [16, k_tau]
        cnt = 2.0 * H[:, :n_oh].sum(axis=1) + val[:, 0]  # [16]
        tail = 0.5 * (val + cnt[:, None])  # [16, k_tau]
        H[:, n_oh:-1] = tail[:, :-1] - tail[:, 1:]
        H[:, -1] = tail[:, -1]
    return H


def run_device(d_pred, d_gt, mask, trace=False, tmpdir=None):
    """Run the SPMD kernel; returns (results_list, exec_time_ns)."""
    Bt = d_pred.shape[0]
    Vx = int(np.prod(d_pred.shape[1:]))
    dp = np.ascontiguousarray(d_pred, dtype=np.float32).reshape(Bt, Vx)
    dg = np.ascontiguousarray(d_gt, dtype=np.float32).reshape(Bt, Vx)
    mm = np.ascontiguousarray(mask, dtype=np.float32).reshape(Bt, Vx)
    half = Vx // 2
    in_maps = []
    for core in range(NCORES):
        bb, h = divmod(core, 2)
        sl = slice(h * half, (h + 1) * half)
        in_maps.append(
            {
                "d_pred": dp[bb, sl].reshape(P, FPP),
                "d_gt": dg[bb, sl].reshape(P, FPP),
                "mask": mm[bb, sl].reshape(P, FPP),
            }
        )
    res = run_bass_kernel_spmd(
        _get_nc(), in_maps, list(range(NCORES)), trace=trace, tmpdir=tmpdir
    )
    return res.results, res.exec_time_ns


def kernel(d_pred, d_gt, mask):
    results, _ = run_device(d_pred, d_gt, mask)
    Bt = d_pred.shape[0]
    loss = 0.0
    for bb in range(Bt):
        e = np.zeros((QW, RW), np.float64)
        msum = 0.0
        for h in range(2):
            r = results[2 * bb + h]
            hp = decode_hist(r["hist_p"])
            hg = decode_hist(r["hist_g"])
            e += hp - hg
            msum += float(hp.sum())
        ed = e.reshape(QW * RW)[:500]
        T = np.cumsum(ed[::-1])[::-1]
        denom = msum + 1e-6
        loss += float(np.sum((T / denom) ** 2))
    loss /= Bt * 500
    return np.float32(loss)
